# revision 20
# baseline (speedup 1.0000x reference)
"""Butterfly sparse-attention MLP kernel for 8 Trainium2 NeuronCores.

Computation (from the reference):
    attn = (w1.T @ w2.T) * sparse_mask          # [4096 s, 4096 t]
    y    = gelu(x @ attn + b2)                  # [8, 768, 4096]

sparse_mask is banded: mask[s, t] == 0 whenever |s - t| > 133.  Each core
owns a 512-wide t-block and needs only a 778-row s-window [t0-133, t0+645)
around it — the exact band, not rounded to chunk alignment.  The window is
6 full 128-row chunks plus a 10-row tail.  Per t-subtile q of 128, the band
covers window rows [128q, 128q+394), i.e. chunks q..q+3 (chunk q+3 only 10
rows deep; mask zeros null the rest), so phase B contracts over <=512 of s.

Phase A computes attn TRANSPOSED, one t-subtile at a time: stationary =
w2.T d-chunk x t-subtile, moving = the 394-wide w1 band slice.  That turns
phase A into 128 long 394-column streams (instead of 224 short ones whose
LDWEIGHTS dominate), then 16 PE transposes restore the [s, t] chunks that
phase B consumes as stationaries.

Sharding: tensor-parallel over t (8 blocks of 512).  All per-core variation
is in the input data (windows are zero-padded at the edges), so one SPMD
BIR serves all 8 cores.

Matmul operands travel as fp16 (values are O(1), accumulation stays fp32
in PSUM); the mask travels as fp8, packed to the in-band columns.  Weight
tensors are host-shuffled into 6-8 KB DMA rows.  Queues: sync/scalar
HW-DGE carry the weights (first-needed chunks issued first) and split the
y stores; gpsimd SW-DGE carries x, paced behind the 6th w1 load; the
vector queue carries the small constants.
"""

import numpy as np

B, T, D = 8, 768, 4096
N = B * T            # 6144 rows of x
NCORES = 8
TB = 512             # t-columns per core
P = 128
MARGIN = 133         # s-window extends this far before/after the t-block
SW = TB + 2 * MARGIN  # 778-row s-window
NCH = 7              # 6 full 128-row chunks + one 10-row tail
CH_ROWS = [128, 128, 128, 128, 128, 128, SW - 6 * P]   # last = 10
BW = 2 * MARGIN + P  # 394-wide s-band per t-subtile
DCH = D // P         # 32 d-chunks (contraction of phase A)
NQ = TB // P         # 4 t-subtiles per core
GN = 1536            # n-group width in phase B
NG = N // GN         # 4 n-groups
MMN = 512            # moving-operand / PSUM-bank free-dim cap per matmul
BANDCH = 4           # s-chunks feeding one t-subtile (covers +-133 band)
W1PACK = 4           # w1 d-chunks packed per DMA row (6.1 KB descriptors)
W2QT = 8             # w2T d-chunks packed per quarter DMA (8 KB rows)

_NC = None
_MASK_FP8 = None     # resolved on first _get_nc(): True -> fp8 mask path


def _band(j):
    """t-column range [lo, hi) of attn chunk j that phase B reads."""
    lo = P * max(0, j - (BANDCH - 1))
    hi = P * min(NQ - 1, j) + P
    return lo, hi


def _build_module(mask_dt_name="float8e4"):
    from concourse import bacc, bass, mybir, tile
    from concourse.tile_rust import add_dep_helper

    f32 = mybir.dt.float32
    f16 = mybir.dt.float16
    mask_dt = getattr(mybir.dt, mask_dt_name)
    PSUM = bass.MemorySpace.PSUM

    nc = bacc.Bacc("TRN2", target_bir_lowering=False, debug=False)
    xT_d = nc.declare_dram_parameter("xT_s", [SW, N], f16, isOutput=False)
    w1_d = nc.declare_dram_parameter(
        "w1_s", [DCH // W1PACK, P, W1PACK * SW], f16, isOutput=False)
    # w2.T in per-t-subtile layout, split into 4 d-quarters:
    # [qt, p, 1024q + 128c + u] = w2T[128*(8qt+c)+p, t0+128q+u]
    w2_d = nc.declare_dram_parameter(
        "w2q_s", [DCH // W2QT, P, NQ * W2QT * P], f16, isOutput=False)
    mask_d = nc.declare_dram_parameter("maskT_s", [P, NQ * BW], mask_dt,
                                       isOutput=False)
    b2_d = nc.declare_dram_parameter("b2c_s", [P, NQ], f32, isOutput=False)
    eye_d = nc.declare_dram_parameter("eye_s", [P, P], f16, isOutput=False)
    yT_d = nc.declare_dram_parameter("yT_s", [TB, N], f16, isOutput=True)

    with tile.TileContext(nc) as tc:
        with (
            tc.tile_pool(name="const", bufs=1) as cpool,
            tc.tile_pool(name="attn", bufs=1) as apool,
            tc.tile_pool(name="xp", bufs=16) as xp,
            tc.tile_pool(name="yp", bufs=4) as yp,
        ):
            # Small constants head the gpsimd queue (x is gated behind w1
            # anyway, so they don't steal early weight bandwidth).
            eye_t = cpool.tile([P, P], f16)
            nc.gpsimd.dma_start(eye_t[:], eye_d[:])
            b2_t = cpool.tile([P, NQ], f32)
            nc.gpsimd.dma_start(b2_t[:], b2_d[:])
            m_t = cpool.tile([P, NQ * BW], mask_dt)
            nc.gpsimd.dma_start(m_t[:], mask_d[:])

            # ---- Phase A: attnT[t, s-band] = (w2T.T @ w1) * maskT --------
            attn_sb = [
                apool.tile([P, TB], f16, name=f"attn_sb{j}")
                for j in range(NCH)
            ]
            # The 394-wide band covers only 10 rows of chunk q+3, but phase B
            # contracts all 128; pre-zero those column regions (the piece
            # copy later overwrites the 10 valid rows — walrus rejects
            # partition-offset memsets, so clear all 128 partitions).
            for q in range(NQ - 1):
                nc.vector.memset(
                    attn_sb[q + BANDCH - 1][:, q * P:(q + 1) * P], 0.0
                )
            w1_insts = []
            with (
                tc.tile_pool(name="w1p", bufs=1) as w1p,
                tc.tile_pool(name="w2p", bufs=1) as w2p,
                tc.tile_pool(name="psA", bufs=1, space=PSUM) as psA,
                tc.tile_pool(name="psT", bufs=2, space=PSUM) as psT,
            ):
                attnT_ps = [
                    psA.tile([P, BW], f32, name=f"attnT_ps{q}")
                    for q in range(NQ)
                ]
                w1_t, w2_t = [None] * (DCH // W1PACK), [None] * (DCH // W2QT)
                # Issue order puts the c=0 operands first on each queue.
                w1_eng = {0: nc.scalar, 1: nc.sync, 2: nc.scalar,
                          3: nc.scalar, 4: nc.sync, 5: nc.scalar,
                          6: nc.sync, 7: nc.scalar}
                issue = [("w2", 0, nc.sync), ("w1", 0, nc.scalar),
                         ("w1", 1, nc.sync), ("w1", 2, nc.scalar),
                         ("w2", 1, nc.sync), ("w1", 3, nc.scalar),
                         ("w1", 4, nc.sync), ("w1", 5, nc.scalar),
                         ("w2", 2, nc.sync), ("w2", 3, nc.scalar),
                         ("w1", 6, nc.sync), ("w1", 7, nc.scalar)]
                for kind, idx, eng in issue:
                    if kind == "w2":
                        w2_t[idx] = w2p.tile([P, NQ * W2QT * P], f16,
                                             name=f"w2_t{idx}")
                        eng.dma_start(w2_t[idx][:], w2_d[idx])
                    else:
                        w1_t[idx] = w1p.tile([P, W1PACK * SW], f16,
                                             name=f"w1_t{idx}")
                        w1_insts.append(
                            eng.dma_start(w1_t[idx][:], w1_d[idx]))
                for c in range(DCH):
                    pi, half = c // W1PACK, c % W1PACK
                    qt, cc = c // W2QT, c % W2QT
                    for q in range(NQ):
                        nc.tensor.matmul(
                            attnT_ps[q][:],
                            w2_t[qt][:, (q * W2QT + cc) * P:
                                     (q * W2QT + cc + 1) * P],
                            w1_t[pi][:, half * SW + q * P:
                                     half * SW + q * P + BW],
                            start=(c == 0),
                            stop=(c == DCH - 1),
                        )
                # mask, transpose back to [s, t] chunks, per q so phase B's
                # first t-subtile unblocks as early as possible
                for q in range(NQ):
                    aT = apool.tile([P, BW], f16, name=f"attnT_sb{q}")
                    nc.vector.tensor_mul(
                        aT[:], attnT_ps[q][:], m_t[:, q * BW:(q + 1) * BW]
                    )
                    for cc in range(BANDCH):
                        w = P if cc < BANDCH - 1 else BW - 3 * P   # 10 tail
                        j = q + cc
                        tp = psT.tile([P, P], f16, name="tp", tag="tp")
                        nc.tensor.transpose(
                            tp[:w, :], aT[:, cc * P:cc * P + w], eye_t[:]
                        )
                        nc.vector.tensor_copy(
                            attn_sb[j][:w, q * P:(q + 1) * P], tp[:w, :]
                        )

            # x rides the gpsimd SW-DGE queue, paced behind the 6th w1 load
            # so early HBM bandwidth goes to the weights, which gate all of
            # phase B.
            x_t = {}
            for g in range(NG):
                for j in range(NCH):
                    r = CH_ROWS[j]
                    xt = xp.tile([P, GN], f16, name="x_t", tag="x_t")
                    xi = nc.gpsimd.dma_start(
                        xt[:r], xT_d[j * P:j * P + r, g * GN:(g + 1) * GN]
                    )
                    if g == 0 and j == 0:
                        add_dep_helper(
                            xi.ins, w1_insts[5].ins,
                            sync=True, reason="pace x behind w1",
                        )
                    x_t[g, j] = xt

            # ---- Phase B: yT[t, n] = gelu(attn.T @ xT + b2) on the band --
            with tc.tile_pool(name="psB", bufs=6, space=PSUM) as psB:
                st = 0
                NH = GN // MMN
                for g in range(NG):
                    for q in range(NQ):
                        y_sb = yp.tile([P, GN], f16, name="y_sb", tag="y_sb")
                        # c outer / h inner: the same attn stationary serves
                        # NH back-to-back matmuls, hiding the LDWEIGHTS.
                        y_pss = [
                            psB.tile([P, MMN], f32, name="y_ps", tag="y_ps")
                            for _ in range(NH)
                        ]
                        for c in range(BANDCH):
                            j = q + c
                            r = CH_ROWS[j]
                            for h in range(NH):
                                nc.tensor.matmul(
                                    y_pss[h][:],
                                    attn_sb[j][:r, q * P:(q + 1) * P],
                                    x_t[g, j][:r, h * MMN:(h + 1) * MMN],
                                    start=(c == 0),
                                    stop=(c == BANDCH - 1),
                                )
                        for h in range(NH):
                            nc.scalar.activation(
                                y_sb[:, h * MMN:(h + 1) * MMN],
                                y_pss[h][:],
                                mybir.ActivationFunctionType.Gelu,
                                bias=b2_t[:, q:q + 1],
                                scale=1.0,
                            )
                        st_eng = nc.sync if st % 2 == 0 else nc.scalar
                        st += 1
                        st_eng.dma_start(
                            yT_d[q * P:(q + 1) * P, g * GN:(g + 1) * GN],
                            y_sb[:],
                        )

    nc.compile()
    nc.finalize()
    return nc


def _mask_np_dtype():
    try:
        import ml_dtypes
        return np.dtype(ml_dtypes.float8_e4m3fn)
    except Exception:
        return None


def _get_nc():
    global _NC, _MASK_FP8
    if _NC is None:
        if _mask_np_dtype() is not None:
            try:
                _NC = _build_module("float8e4")
                _MASK_FP8 = True
            except Exception:
                _NC = _build_module("float16")
                _MASK_FP8 = False
        else:
            _NC = _build_module("float16")
            _MASK_FP8 = False
    return _NC


def prepare_in_maps(x, w1, w2, b2, sparse_mask):
    x = np.asarray(x, dtype=np.float32)
    w1 = np.asarray(w1, dtype=np.float32)
    w2 = np.asarray(w2, dtype=np.float32)
    b2 = np.asarray(b2, dtype=np.float32)
    sparse_mask = np.asarray(sparse_mask, dtype=np.float32)

    xT = np.ascontiguousarray(x.reshape(N, D).T.astype(np.float16))   # [s, n]
    w2T = np.ascontiguousarray(w2.T.astype(np.float16))               # [d, t]

    # Zero-pad the s axis by MARGIN on both sides so every core's window is
    # a plain slice; mask zeros make the padded rows contribute nothing.
    xT_pad = np.zeros((D + 2 * MARGIN, N), dtype=np.float16)
    xT_pad[MARGIN:MARGIN + D] = xT
    w1_pad = np.zeros((D, D + 2 * MARGIN), dtype=np.float16)
    w1_pad[:, MARGIN:MARGIN + D] = w1.astype(np.float16)
    mask_pad = np.zeros((D + 2 * MARGIN, D), dtype=np.float16)
    mask_pad[MARGIN:MARGIN + D] = sparse_mask.astype(np.float16)

    mdt = _mask_np_dtype()
    eye = np.eye(P, dtype=np.float16)
    in_maps = []
    for i in range(NCORES):
        s0 = i * TB           # window start in padded coords
        t0 = i * TB
        w1win = w1_pad[:, s0:s0 + SW]                     # [D, SW]
        w1_s = (w1win.reshape(DCH // W1PACK, W1PACK, P, SW)
                .transpose(0, 2, 1, 3)
                .reshape(DCH // W1PACK, P, W1PACK * SW))
        w2win = w2T[:, t0:t0 + TB]                        # [D, TB]
        # [qt, p, (q*8 + c)*128 + u] = w2win[128*(8qt+c)+p, 128q+u]
        w2_s = (w2win.reshape(DCH // W2QT, W2QT, P, NQ, P)
                .transpose(0, 2, 3, 1, 4)
                .reshape(DCH // W2QT, P, NQ * W2QT * P))
        mwin = mask_pad[s0:s0 + SW, t0:t0 + TB]           # [SW, TB]
        # transposed band-packed mask: [p, q*BW + u] = mwin[128q+u, 128q+p]
        m_s = np.zeros((P, NQ * BW), dtype=np.float16)
        for q in range(NQ):
            m_s[:, q * BW:(q + 1) * BW] = mwin[q * P:q * P + BW,
                                               q * P:(q + 1) * P].T
        if mdt is not None and _MASK_FP8:
            m_s = m_s.astype(mdt)
        in_maps.append({
            "xT_s": np.ascontiguousarray(xT_pad[s0:s0 + SW]),
            "w1_s": np.ascontiguousarray(w1_s),
            "w2q_s": np.ascontiguousarray(w2_s),
            "maskT_s": np.ascontiguousarray(m_s),
            "b2c_s": np.ascontiguousarray(b2[t0:t0 + TB].reshape(NQ, P).T),
            "eye_s": eye,
        })
    return in_maps


def assemble(results):
    out = np.empty((N, D), dtype=np.float32)
    for i in range(NCORES):
        out[:, i * TB:(i + 1) * TB] = results[i]["yT_s"].T.astype(np.float32)
    return out.reshape(B, T, D)


def _band_ok(sparse_mask):
    """The Bass kernel only computes attn where each core's 4-chunk window
    covers the mask; verify every mask nonzero falls inside that region."""
    s_idx, t_idx = np.nonzero(np.asarray(sparse_mask) != 0)
    if len(s_idx) == 0:
        return True
    w0 = (t_idx // TB) * TB - MARGIN          # per-core s-window start
    j = (s_idx - w0) // P                     # s-chunk within window
    q = (t_idx % TB) // P                     # t-subtile
    return bool(np.all((j >= q) & (j <= q + BANDCH - 1)
                       & (s_idx >= w0) & (s_idx < w0 + SW)))


def _reference_fallback(x, w1, w2, b2, sparse_mask):
    import jax
    import jax.numpy as jnp

    cpu = jax.devices("cpu")[0]
    with jax.default_device(cpu):
        attn = jnp.einsum("ds,td->st", jnp.asarray(w1), jnp.asarray(w2))
        attn = attn * jnp.asarray(sparse_mask)
        y = jnp.einsum("bds,st->bdt", jnp.asarray(x), attn) + jnp.asarray(b2)
        return np.asarray(jax.nn.gelu(y, approximate=False), dtype=np.float32)


def kernel(x, w1, w2, b2, sparse_mask):
    import time

    from concourse.bass_utils import run_bass_kernel_spmd

    if (np.shape(x) != (B, T, D) or np.shape(w1) != (D, D)
            or np.shape(w2) != (D, D) or np.shape(b2) != (D,)
            or np.shape(sparse_mask) != (D, D) or not _band_ok(sparse_mask)):
        return _reference_fallback(x, w1, w2, b2, sparse_mask)

    nc = _get_nc()           # resolves the mask dtype before prepare
    in_maps = prepare_in_maps(x, w1, w2, b2, sparse_mask)
    last_err = None
    for attempt in range(3):
        try:
            res = run_bass_kernel_spmd(nc, in_maps, list(range(NCORES)))
            return assemble(res.results)
        except Exception as e:  # transient NRT/device errors: retry
            last_err = e
            time.sleep(2.0 * (attempt + 1))
    raise last_err


# revision 21
# speedup vs baseline: 1.1485x; 1.1485x over previous
"""Butterfly sparse-attention MLP kernel for 8 Trainium2 NeuronCores.

Computation (from the reference):
    attn = (w1.T @ w2.T) * sparse_mask          # [4096 s, 4096 t]
    y    = gelu(x @ attn + b2)                  # [8, 768, 4096]

sparse_mask is banded: mask[s, t] == 0 whenever |s - t| > 133.  Each core
owns a 512-wide t-block and only needs an 896-wide s-window around it.
Per t-subtile of 128, only 4 of the 7 s-chunks in the window can carry
non-zero attn, so phase B contracts over 512 of s instead of 4096, and
phase A only computes the in-band t-columns of each attn chunk.

Sharding: tensor-parallel over t (8 blocks of 512).  All per-core variation
is in the input data (windows are zero-padded at the edges; mask zeros make
padded contributions exactly zero), so one SPMD BIR serves all 8 cores.

Matmul operands travel as fp16 (10-bit mantissa; values here are O(1), and
accumulation stays fp32 in PSUM) which halves HBM traffic.  Weight tensors
are host-shuffled so each DMA descriptor is 3.5-4 KB — the HW-DGE queues
are descriptor-rate limited (~60 M/s), not byte limited.  Streams are
spread over the sync/scalar HW-DGE queues plus the gpsimd SW-DGE queue.
"""

import numpy as np

B, T, D = 8, 768, 4096
N = B * T            # 6144 rows of x
NCORES = 8
TB = 512             # t-columns per core
P = 128
MARGIN = 192         # s-window extends this far before/after the t-block
SW = TB + 2 * MARGIN  # 896 s-window width
NCH = SW // P        # 7 s-chunks
DCH = D // P         # 32 d-chunks (contraction of phase A)
NQ = TB // P         # 4 t-subtiles per core
GN = 2048            # n-group width in phase B
NG = N // GN         # 3 n-groups
MMN = 512            # moving-operand / PSUM-bank free-dim cap per matmul
BANDCH = 4           # s-chunks feeding one t-subtile (covers +-133 band)
W1PACK = 2           # w1 d-chunks packed per DMA row (3.5 KB descriptors)
W2PACK = 4           # w2T d-chunks packed per DMA row (4 KB descriptors)

_NC = None


def _band(j):
    """t-column range [lo, hi) of attn chunk j that phase B reads."""
    lo = P * max(0, j - (BANDCH - 1))
    hi = P * min(NQ - 1, j) + P
    return lo, hi


def _build_module():
    from concourse import bacc, bass, mybir, tile
    from concourse.tile_rust import add_dep_helper

    f32 = mybir.dt.float32
    f16 = mybir.dt.float16
    PSUM = bass.MemorySpace.PSUM

    nc = bacc.Bacc("TRN2", target_bir_lowering=False, debug=False)
    xT_d = nc.declare_dram_parameter("xT_s", [NCH, P, N], f16, isOutput=False)
    w1_d = nc.declare_dram_parameter(
        "w1_s", [DCH // W1PACK, P, W1PACK * SW], f16, isOutput=False)
    w2T_d = nc.declare_dram_parameter(
        "w2T_s", [DCH // W2PACK, P, W2PACK * TB], f16, isOutput=False)
    mask_d = nc.declare_dram_parameter("mask_s", [SW, TB], f16, isOutput=False)
    b2_d = nc.declare_dram_parameter("b2c_s", [P, NQ], f32, isOutput=False)
    yT_d = nc.declare_dram_parameter("yT_s", [TB, N], f16, isOutput=True)

    with tile.TileContext(nc) as tc:
        with (
            tc.tile_pool(name="const", bufs=1) as cpool,
            tc.tile_pool(name="attn", bufs=1) as apool,
            tc.tile_pool(name="mp", bufs=1) as mp,
            tc.tile_pool(name="xp", bufs=NG * NCH) as xp,
            tc.tile_pool(name="yp", bufs=6) as yp,
        ):
            b2_t = cpool.tile([P, NQ], f32)
            nc.gpsimd.dma_start(b2_t[:], b2_d[:])

            # Masks land early via the (otherwise idle) SW-DGE queue.
            m_ts = []
            for j in range(NCH):
                m_t = mp.tile([P, TB], f16, name=f"m_t{j}")
                nc.gpsimd.dma_start(m_t[:], mask_d[j * P:(j + 1) * P, :])
                m_ts.append(m_t)

            engs = [nc.sync, nc.scalar, nc.gpsimd]

            # ---- Phase A: attn[s, t] = (w1.T @ w2T) * mask on the band ----
            attn_sb = []
            w1_insts = []
            with (
                tc.tile_pool(name="w1p", bufs=8) as w1p,
                tc.tile_pool(name="w2p", bufs=4) as w2p,
                tc.tile_pool(name="psA", bufs=1, space=PSUM) as psA,
            ):
                attn_ps = [
                    psA.tile([P, TB], f32, name=f"attn_ps{j}") for j in range(NCH)
                ]
                for bb in range(DCH // W2PACK):
                    w2_t = w2p.tile([P, W2PACK * TB], f16)
                    nc.scalar.dma_start(w2_t[:], w2T_d[bb])
                    for hb in range(W2PACK // W1PACK):
                        pi = bb * (W2PACK // W1PACK) + hb
                        w1_t = w1p.tile([P, W1PACK * SW], f16)
                        w1_insts.append(nc.sync.dma_start(w1_t[:], w1_d[pi]))
                        for half in range(W1PACK):
                            k = bb * W2PACK + hb * W1PACK + half
                            w1sl = w1_t[:, half * SW:(half + 1) * SW]
                            w2sl = w2_t[:, (hb * W1PACK + half) * TB:
                                        (hb * W1PACK + half + 1) * TB]
                            for j in (3, 2, 4, 1, 5, 0, 6):
                                lo, hi = _band(j)
                                nc.tensor.matmul(
                                    attn_ps[j][:, lo:hi],
                                    w1sl[:, j * P:(j + 1) * P],
                                    w2sl[:, lo:hi],
                                    start=(k == 0),
                                    stop=(k == DCH - 1),
                                )
                for j in range(NCH):
                    lo, hi = _band(j)
                    a_t = apool.tile([P, TB], f16, name=f"attn_sb{j}")
                    nc.vector.tensor_mul(
                        a_t[:, lo:hi], attn_ps[j][:, lo:hi], m_ts[j][:, lo:hi]
                    )
                    attn_sb.append(a_t)

            # ---- Phase B: yT[t, n] = gelu(attn.T @ xT + b2) on the band ----
            with tc.tile_pool(name="psB", bufs=4, space=PSUM) as psB:
                for g in range(NG):
                    x_t = []
                    gate = {0: 11, 1: 13, 2: 15}[g]
                    for j in range(NCH):
                        xt = xp.tile([P, GN], f16, name="x_t", tag="x_t")
                        xi = nc.gpsimd.dma_start(
                            xt[:], xT_d[j, :, g * GN:(g + 1) * GN]
                        )
                        add_dep_helper(
                            xi.ins, w1_insts[gate].ins,
                            sync=True, reason="pace x prefetch behind w1",
                        )
                        x_t.append(xt)
                    for q in range(NQ):
                        for h in range(GN // (2 * MMN)):
                            y_ps = psB.tile([P, 2 * MMN], f32, name="y_ps",
                                            tag="y_ps")
                            for hh in range(2):
                                osl = slice(hh * MMN, (hh + 1) * MMN)
                                nsl = slice((2 * h + hh) * MMN,
                                            (2 * h + hh + 1) * MMN)
                                for c in range(BANDCH):
                                    j = q + c
                                    nc.tensor.matmul(
                                        y_ps[:, osl],
                                        attn_sb[j][:, q * P:(q + 1) * P],
                                        x_t[j][:, nsl],
                                        start=(c == 0),
                                        stop=(c == BANDCH - 1),
                                    )
                            y_sb = yp.tile([P, 2 * MMN], f16, name="y_sb",
                                           tag="y_sb")
                            nc.scalar.activation(
                                y_sb[:],
                                y_ps[:],
                                mybir.ActivationFunctionType.Gelu,
                                bias=b2_t[:, q:q + 1],
                                scale=1.0,
                            )
                            st_eng = nc.sync if (q + h) % 2 == 0 else nc.scalar
                            st_eng.dma_start(
                                yT_d[q * P:(q + 1) * P,
                                     g * GN + 2 * h * MMN:
                                     g * GN + 2 * (h + 1) * MMN],
                                y_sb[:],
                            )

    nc.compile()
    nc.finalize()
    return nc


def _get_nc():
    global _NC
    if _NC is None:
        _NC = _build_module()
    return _NC


def prepare_in_maps(x, w1, w2, b2, sparse_mask):
    x = np.asarray(x, dtype=np.float32)
    w1 = np.asarray(w1, dtype=np.float32)
    w2 = np.asarray(w2, dtype=np.float32)
    b2 = np.asarray(b2, dtype=np.float32)
    sparse_mask = np.asarray(sparse_mask, dtype=np.float32)

    xT = np.ascontiguousarray(x.reshape(N, D).T.astype(np.float16))   # [s, n]
    w2T = np.ascontiguousarray(w2.T.astype(np.float16))               # [d, t]

    # Zero-pad the s axis by MARGIN on both sides so every core's window is
    # a plain slice; mask zeros make the padded rows contribute nothing.
    xT_pad = np.zeros((D + 2 * MARGIN, N), dtype=np.float16)
    xT_pad[MARGIN:MARGIN + D] = xT
    w1_pad = np.zeros((D, D + 2 * MARGIN), dtype=np.float16)
    w1_pad[:, MARGIN:MARGIN + D] = w1.astype(np.float16)
    mask_pad = np.zeros((D + 2 * MARGIN, D), dtype=np.float16)
    mask_pad[MARGIN:MARGIN + D] = sparse_mask.astype(np.float16)

    in_maps = []
    for i in range(NCORES):
        s0 = i * TB           # window start in padded coords
        t0 = i * TB
        w1win = w1_pad[:, s0:s0 + SW]                     # [D, SW]
        # pack W1PACK d-chunks per DMA row: [DCH/W1PACK, P, W1PACK*SW]
        w1_s = (w1win.reshape(DCH // W1PACK, W1PACK, P, SW)
                .transpose(0, 2, 1, 3)
                .reshape(DCH // W1PACK, P, W1PACK * SW))
        w2win = w2T[:, t0:t0 + TB]                        # [D, TB]
        w2_s = (w2win.reshape(DCH // W2PACK, W2PACK, P, TB)
                .transpose(0, 2, 1, 3)
                .reshape(DCH // W2PACK, P, W2PACK * TB))
        in_maps.append({
            "xT_s": np.ascontiguousarray(
                xT_pad[s0:s0 + SW].reshape(NCH, P, N)),
            "w1_s": np.ascontiguousarray(w1_s),
            "w2T_s": np.ascontiguousarray(w2_s),
            "mask_s": np.ascontiguousarray(mask_pad[s0:s0 + SW, t0:t0 + TB]),
            "b2c_s": np.ascontiguousarray(b2[t0:t0 + TB].reshape(NQ, P).T),
        })
    return in_maps


def assemble(results):
    out = np.empty((N, D), dtype=np.float32)
    for i in range(NCORES):
        out[:, i * TB:(i + 1) * TB] = results[i]["yT_s"].T.astype(np.float32)
    return out.reshape(B, T, D)


def _band_ok(sparse_mask):
    """The Bass kernel only computes attn where each core's 4-chunk window
    covers the mask; verify every mask nonzero falls inside that region."""
    s_idx, t_idx = np.nonzero(np.asarray(sparse_mask) != 0)
    if len(s_idx) == 0:
        return True
    w0 = (t_idx // TB) * TB - MARGIN          # per-core s-window start
    j = (s_idx - w0) // P                     # s-chunk within window
    q = (t_idx % TB) // P                     # t-subtile
    return bool(np.all((j >= q) & (j <= q + BANDCH - 1)
                       & (s_idx >= w0) & (s_idx < w0 + SW)))


def _reference_fallback(x, w1, w2, b2, sparse_mask):
    import jax
    import jax.numpy as jnp

    cpu = jax.devices("cpu")[0]
    with jax.default_device(cpu):
        attn = jnp.einsum("ds,td->st", jnp.asarray(w1), jnp.asarray(w2))
        attn = attn * jnp.asarray(sparse_mask)
        y = jnp.einsum("bds,st->bdt", jnp.asarray(x), attn) + jnp.asarray(b2)
        return np.asarray(jax.nn.gelu(y, approximate=False), dtype=np.float32)


def kernel(x, w1, w2, b2, sparse_mask):
    import time

    from concourse.bass_utils import run_bass_kernel_spmd

    if (np.shape(x) != (B, T, D) or np.shape(w1) != (D, D)
            or np.shape(w2) != (D, D) or np.shape(b2) != (D,)
            or np.shape(sparse_mask) != (D, D) or not _band_ok(sparse_mask)):
        return _reference_fallback(x, w1, w2, b2, sparse_mask)

    in_maps = prepare_in_maps(x, w1, w2, b2, sparse_mask)
    nc = _get_nc()
    last_err = None
    for attempt in range(3):
        try:
            res = run_bass_kernel_spmd(nc, in_maps, list(range(NCORES)))
            return assemble(res.results)
        except Exception as e:  # transient NRT/device errors: retry
            last_err = e
            time.sleep(2.0 * (attempt + 1))
    raise last_err
